# revision 1
# baseline (speedup 1.0000x reference)
"""Trainium2 Bass kernel for the dense RandLA-Net block.

Reference computation (per batch b, point n, K=16 neighbors):
    enc   = [center(3), npos(3), rel(3), dist(1)]            # 10 dims
    rp    = relu(enc @ W_rel + b_rel)                        # 64
    f     = [rp, nfeat]                                      # 128
    att   = softmax_k(f @ W_att)                             # 128
    agg   = sum_k f * att                                    # 128
    out   = relu(agg @ W_glob + b_glob)                      # 128

Sharding: 8 cores = 4 batches x 2 point-halves (8192 points/core).
Within a core the 131072 (point, k) pairs are processed channel-major in
"block-k-major" column order: 16 blocks of 512 points, 16 k-slabs of 512
columns each.  Geometry (center/npos/dist) is computed with a grouped
GPSIMD ap_gather layout; neighbor features come from an SBUF-source
DMA transpose-gather; rp is one K=7 matmul per 512-column chunk against a
packed "enc" tile (rel is algebraically folded:  Wc*center + Wn*npos +
Wr*(npos-center) = (Wc-Wr)*center + (Wn+Wr)*npos).  The softmax-weighted
sums over k are PSUM-accumulated identity matmuls over the 16 k-slabs.
"""

import os
import sys

import numpy as np

sys.path.insert(0, "/opt/trn_rl_repo")

import ml_dtypes

import concourse.bass as bass
import concourse.tile as tile
from concourse import mybir, bacc
from concourse.bass_utils import run_bass_kernel_spmd

F32 = mybir.dt.float32
BF16 = mybir.dt.bfloat16
I16 = mybir.dt.int16
AF = mybir.ActivationFunctionType
OP = mybir.AluOpType
BF = ml_dtypes.bfloat16

B, C_IN, N, K = 4, 64, 16384, 16
D_REL, C_MID, C_OUT = 64, 128, 128
NP = N // 2            # points per core
PK = NP * K            # columns per core (131072)
NT = 16                # F tiles (= point blocks of 512)
LT = PK // NT          # 8192 cols per tile
NCH = 16               # chunks per tile
LC = 512               # chunk cols
ENC_F = PK // 4        # packed enc free size (32768)


def _build_kernel():
    nc = bacc.Bacc("TRN2", target_bir_lowering=False)

    # ---- DRAM tensors (per-core inputs) ----
    tabX = nc.dram_tensor("tabX", [128, N], F32, kind="ExternalInput")       # x duplicated (2-pack)
    tabG = nc.dram_tensor("tabG", [128, N], F32, kind="ExternalInput")       # grouped pos table
    posG = nc.dram_tensor("posG", [128, NP // 8], F32, kind="ExternalInput") # per-group own points
    gidxN = nc.dram_tensor("gidxN", [128, 1024], I16, kind="ExternalInput")  # geometry idx (wrapped)
    nfidx = nc.dram_tensor("nfidx", [128, 4096], I16, kind="ExternalInput")  # feature idx (2-pack)
    wattsw = nc.dram_tensor("wattsw", [128, 128], BF16, kind="ExternalInput")
    pswap = nc.dram_tensor("pswap", [128, 128], BF16, kind="ExternalInput")
    w7x4 = nc.dram_tensor("w7x4", [128, 64], BF16, kind="ExternalInput")
    watt = nc.dram_tensor("watt", [128, 128], BF16, kind="ExternalInput")
    wglob = nc.dram_tensor("wglob", [128, 128], BF16, kind="ExternalInput")
    ident = nc.dram_tensor("ident", [128, 128], BF16, kind="ExternalInput")
    sel3 = nc.dram_tensor("sel3", [128, 128], BF16, kind="ExternalInput")
    brel = nc.dram_tensor("brel", [128, 1], F32, kind="ExternalInput")
    bglob = nc.dram_tensor("bglob", [128, 1], F32, kind="ExternalInput")
    outp = nc.dram_tensor("outp", [128, NP], F32, kind="ExternalOutput")

    with tile.TileContext(nc) as tc:
        with tc.tile_pool(name="persist", bufs=1) as pp:
            enc = pp.tile([128, ENC_F], BF16)        # packed enc: subtile q at parts 32q..32q+6
            posG_sb = pp.tile([128, NP // 8], F32)
            w7_sb = pp.tile([128, 64], BF16)
            watt_sb = pp.tile([128, 128], BF16)
            wattsw_sb = pp.tile([128, 128], BF16)
            pswap_sb = pp.tile([128, 128], BF16)
            nc.sync.dma_start(out=wattsw_sb, in_=wattsw.ap())
            nc.sync.dma_start(out=pswap_sb, in_=pswap.ap())
            wglob_sb = pp.tile([128, 128], BF16)
            ident_sb = pp.tile([128, 128], BF16)
            sel3_sb = pp.tile([128, 128], BF16)
            brel_sb = pp.tile([128, 1], F32)
            bglob_sb = pp.tile([128, 1], F32)
            nc.sync.dma_start(out=posG_sb, in_=posG.ap())
            nc.sync.dma_start(out=w7_sb, in_=w7x4.ap())
            nc.sync.dma_start(out=watt_sb, in_=watt.ap())
            nc.sync.dma_start(out=wglob_sb, in_=wglob.ap())
            nc.sync.dma_start(out=ident_sb, in_=ident.ap())
            nc.sync.dma_start(out=sel3_sb, in_=sel3.ap())
            nc.sync.dma_start(out=brel_sb, in_=brel.ap())
            nc.sync.dma_start(out=bglob_sb, in_=bglob.ap())

            # ================= Phase B: geometry =================
            with tc.tile_pool(name="geo", bufs=1) as gp, \
                 tc.tile_pool(name="geops", bufs=2, space="PSUM") as gpsum:
                tabG_sb = gp.tile([128, N], F32)
                gidx_sb = gp.tile([128, 1024], I16)
                nc.sync.dma_start(out=tabG_sb, in_=tabG.ap())
                nc.sync.dma_start(out=gidx_sb, in_=gidxN.ap())
                for h in range(2):
                    gN = gp.tile([128, LT], F32, tag="gN")
                    nc.gpsimd.ap_gather(
                        out_ap=gN[:, :], in_ap=tabG_sb[:, :],
                        idxs_ap=gidx_sb[:, h * 512:(h + 1) * 512],
                        channels=128, num_elems=N, d=1, num_idxs=LT)
                    # s = npos - center   (center broadcast over k)
                    cen = posG_sb[:, h * 512:h * 512 + 512]
                    cen_b = bass.AP(tensor=cen.tensor, offset=cen.offset,
                                    ap=[[cen.ap[0][0], 128], [0, 16], [1, 512]])
                    s_t = gp.tile([128, LT], BF16, tag="s")
                    nc.vector.tensor_tensor(
                        out=s_t.rearrange("p (k i) -> p k i", i=512),
                        in0=gN.rearrange("p (k i) -> p k i", i=512),
                        in1=cen_b, op=OP.subtract)
                    m2 = s_t
                    nc.vector.tensor_mul(m2, s_t, s_t)
                    cenrep = gp.tile([128, 2048], F32, tag="cenrep")
                    crsrc = posG_sb[:, h * 512:h * 512 + 512]
                    crin = bass.AP(tensor=crsrc.tensor, offset=crsrc.offset,
                                   ap=[[crsrc.ap[0][0], 128], [0, 4], [1, 512]])
                    nc.vector.tensor_copy(
                        cenrep.rearrange("p (a i) -> p a i", i=512), crin)
                    dsb = gp.tile([128, LT], BF16, tag="dsb")
                    for cc in range(16):
                        psd = gpsum.tile([128, 512], F32, tag="psd")
                        nc.tensor.matmul(psd, sel3_sb, m2[:, cc * 512:(cc + 1) * 512],
                                         start=True, stop=True)
                        nc.scalar.activation(out=dsb[:, cc * 512:(cc + 1) * 512],
                                             in_=psd, func=AF.Sqrt)
                    # assembly DMAs into packed enc
                    for g in range(8):
                        ebase = g * 4096 + h * 2048
                        for r in range(4):
                            # center rows 32r..32r+2 (one contiguous cast DMA)
                            nc.gpsimd.dma_start(
                                out=enc[32 * r:32 * r + 3, ebase:ebase + 2048],
                                in_=cenrep[16 * g:16 * g + 3, :])
                            # npos rows 32r+3..32r+5
                            src_n = gN[16 * g:16 * g + 3, r * 512:r * 512 + 512]
                            src_n = bass.AP(tensor=src_n.tensor, offset=src_n.offset,
                                            ap=[[src_n.ap[0][0], 3], [2048, 4], [1, 512]])
                            dst_n = enc[32 * r + 3:32 * r + 6, ebase:ebase + 2048]
                            dst_n = bass.AP(tensor=dst_n.tensor, offset=dst_n.offset,
                                            ap=[[dst_n.ap[0][0], 3], [512, 4], [1, 512]])
                            nc.gpsimd.dma_start(out=dst_n, in_=src_n)
                            # dist row 32r+6
                            src_d = dsb[16 * g + 6:16 * g + 7, r * 512:r * 512 + 512]
                            src_d = bass.AP(tensor=src_d.tensor, offset=src_d.offset,
                                            ap=[[src_d.ap[0][0], 1], [2048, 4], [1, 512]])
                            dst_d = enc[32 * r + 6:32 * r + 7, ebase:ebase + 2048]
                            dst_d = bass.AP(tensor=dst_d.tensor, offset=dst_d.offset,
                                            ap=[[dst_d.ap[0][0], 1], [512, 4], [1, 512]])
                            nc.sync.dma_start(out=dst_d, in_=src_d)

            # ================= Phase C: main loop =================
            with tc.tile_pool(name="main", bufs=1) as mp, \
                 tc.tile_pool(name="ftiles", bufs=2) as fp, \
                 tc.tile_pool(name="chunks", bufs=2) as cp, \
                 tc.tile_pool(name="mps", bufs=2, space="PSUM") as mpsum, \
                 tc.tile_pool(name="accps", bufs=1, space="PSUM") as apsum:
                tabX_sb = mp.tile([128, N], F32)
                nfidx_sb = mp.tile([128, 4096], I16)
                nc.sync.dma_start(out=tabX_sb, in_=tabX.ap())
                nc.sync.dma_start(out=nfidx_sb, in_=nfidx.ap())

                for t in range(NT):
                    ft = fp.tile([128, LT], BF16, tag="ft")
                    for hf in range(2):
                        gX = fp.tile([128, 2048], F32, tag="gX")
                        nc.gpsimd.ap_gather(
                            out_ap=gX[:, :], in_ap=tabX_sb[:, :],
                            idxs_ap=nfidx_sb[:, t * 256 + hf * 128:t * 256 + (hf + 1) * 128],
                            channels=128, num_elems=N, d=1, num_idxs=2048)
                        # rows 0-63: swapped half (ft cols 0:4096); rows 64-127: canonical
                        nc.gpsimd.dma_start(
                            out=ft[0:64, hf * 2048:(hf + 1) * 2048], in_=gX[0:64, :])
                        nc.gpsimd.dma_start(
                            out=ft[64:128, 4096 + hf * 2048:4096 + (hf + 1) * 2048],
                            in_=gX[64:128, :])
                    if True:
                        ps_den = apsum.tile([128, 512], F32, tag="den")
                        ps_num = apsum.tile([128, 512], F32, tag="num")
                        for pr in range(NCH // 2):
                            swapped = pr < 4
                            rbase = 64 if swapped else 0
                            widt = wattsw_sb if swapped else watt_sb
                            pacc = pswap_sb if swapped else ident_sb
                            ps_s = mpsum.tile([128, 1024], F32, tag="sc")
                            pcols = slice(pr * 1024, (pr + 1) * 1024)
                            for ci in range(2):
                                cc = 2 * pr + ci
                                q = cc % 4
                                eoff = (t * 4 + cc // 4) * 512
                                cols = slice(cc * 512, (cc + 1) * 512)
                                ps_rp = mpsum.tile([128, 512], F32, tag="rp")
                                nc.tensor.matmul(ps_rp[rbase:rbase + 64, :],
                                                 w7_sb[32 * q:32 * q + 7, :],
                                                 enc[32 * q:32 * q + 7, eoff:eoff + 512],
                                                 start=True, stop=True,
                                                 tile_position=(32 * q, rbase))
                                if cc % 2 == 0:
                                    nc.scalar.activation(out=ft[rbase:rbase + 64, cols],
                                                         in_=ps_rp[rbase:rbase + 64, :],
                                                         func=AF.Relu,
                                                         bias=brel_sb[rbase:rbase + 64, :],
                                                         scale=1.0)
                                else:
                                    nc.vector.tensor_scalar(out=ft[rbase:rbase + 64, cols],
                                                            in0=ps_rp[rbase:rbase + 64, :],
                                                            scalar1=brel_sb[rbase:rbase + 64, :],
                                                            scalar2=0.0,
                                                            op0=OP.add, op1=OP.max)
                                nc.tensor.matmul(ps_s[:, ci * 512:(ci + 1) * 512],
                                                 widt, ft[:, cols],
                                                 start=True, stop=True)
                            eu = cp.tile([128, 2048], BF16, tag="eu")
                            nc.scalar.activation(out=eu[:, 0:1024], in_=ps_s, func=AF.Exp)
                            nc.vector.tensor_mul(eu[:, 1024:2048], ft[:, pcols],
                                                 eu[:, 0:1024])
                            for ci in range(2):
                                cc = 2 * pr + ci
                                nc.tensor.matmul(ps_den, pacc,
                                                 eu[:, ci * 512:(ci + 1) * 512],
                                                 start=(cc == 0), stop=(cc == NCH - 1),
                                                 skip_group_check=True)
                                nc.tensor.matmul(ps_num, pacc,
                                                 eu[:, 1024 + ci * 512:1024 + (ci + 1) * 512],
                                                 start=(cc == 0), stop=(cc == NCH - 1),
                                                 skip_group_check=True)
                        rcp = cp.tile([128, 512], F32, tag="rcp")
                        nc.vector.reciprocal(rcp, ps_den)
                        agg = cp.tile([128, 512], BF16, tag="agg")
                        nc.vector.tensor_mul(agg, ps_num, rcp)
                        ps_o = mpsum.tile([128, 512], F32, tag="rp")
                        nc.tensor.matmul(ps_o, wglob_sb, agg, start=True, stop=True)
                        osb = cp.tile([128, 512], F32, tag="osb")
                        nc.scalar.activation(out=osb, in_=ps_o, func=AF.Relu,
                                             bias=bglob_sb, scale=1.0)
                        nc.sync.dma_start(out=outp.ap()[:, t * 512:(t + 1) * 512], in_=osb)
    nc.compile()
    return nc


_NC = None


def _get_nc():
    global _NC
    if _NC is None:
        _NC = _build_kernel()
    return _NC


def _prep_core(core, x, pos, neigh, Wc, Wn, Wr, wd, W_att, W_glob, b_rel, b_glob):
    b = core // 2
    half = core % 2
    P0 = half * NP
    nb = neigh[b][P0:P0 + NP].astype(np.int64)      # [NP, K]
    xb = x[b]                                        # [64, N] f32
    posb = pos[b]                                    # [N, 3] f32

    # feature table: x duplicated on both partition halves
    tabX = np.concatenate([xb, xb], axis=0).astype(np.float32)   # [128, N]

    # tabG: rows 16g+j (j<3) = pos component j
    tabG = np.zeros((128, N), np.float32)
    for j in range(3):
        tabG[j::16, :] = posb[:, j][None, :]
    # posG: [16g+j, c] = pos comp j of point P0 + g*1024 + c
    posG = np.zeros((128, NP // 8), np.float32)
    pl = posb[P0:P0 + NP]
    for g in range(8):
        for j in range(3):
            posG[16 * g + j] = pl[g * 1024:(g + 1) * 1024, j]

    # geometry idx: block t2 = 2g+h; j in [0, 8192): k = j//512, i = j%512
    A = nb.reshape(16, 512, 16)                      # [block, i, k]
    V = A.transpose(0, 2, 1).reshape(16, LT)         # [block, j] j = k*512+i
    V2 = V.reshape(16, 512, 16).transpose(0, 2, 1)   # [block, j%16, j//16]
    gidxN = np.zeros((128, 1024), np.int16)
    for g in range(8):
        gidxN[16 * g:16 * g + 16, 0:512] = V2[2 * g]
        gidxN[16 * g:16 * g + 16, 512:1024] = V2[2 * g + 1]

    # nfeat idx (4-pack): inst t2, group g gathers chunk m=g//2 of its 16384-col range
    cs = np.arange(PK)
    t_ = cs >> 13
    k_ = (cs >> 9) & 15
    i_ = cs & 511
    s_nf = nb[t_ * 512 + i_, k_]
    nfidx = np.zeros((128, 4096), np.int16)
    for t in range(16):
        for hf in range(2):
            for g in range(8):
                m = g // 4
                base = t * 8192 + m * 4096 + hf * 2048
                seg = s_nf[base:base + 2048]
                nfidx[16 * g:16 * g + 16,
                      t * 256 + hf * 128:t * 256 + (hf + 1) * 128] = \
                    seg.reshape(128, 16).T.astype(np.int16)

    perm = (np.arange(128) + 64) % 128
    w7 = np.concatenate([Wc - Wr, Wn + Wr, wd], axis=0)  # [7, 64]
    w7x4 = np.zeros((128, 64), dtype=BF)
    for q in range(4):
        w7x4[32 * q:32 * q + 7] = w7.astype(BF)
    sel3 = np.zeros((128, 128), dtype=BF)
    for g in range(8):
        for j in range(3):
            sel3[16 * g + j, 16 * g + 6] = 1
    ident = np.eye(128, dtype=BF)

    return {
        "tabX": tabX, "tabG": tabG, "posG": posG,
        "gidxN": gidxN, "nfidx": nfidx,
        "w7x4": w7x4, "watt": W_att.astype(BF), "wglob": W_glob.astype(BF),
        "ident": ident, "sel3": sel3,
        "wattsw": W_att[np.ix_(perm, perm)].astype(BF),
        "pswap": np.roll(np.eye(128, dtype=np.float32), 64, axis=0).astype(BF),
        "brel": np.concatenate([b_rel, b_rel]).reshape(128, 1).astype(np.float32),
        "bglob": b_glob.reshape(128, 1).astype(np.float32),
    }


def kernel(x, pos, neigh_idx, W_rel, b_rel, W_att, W_glob, b_glob, **kw):
    x = np.ascontiguousarray(np.asarray(x, dtype=np.float32))
    pos = np.ascontiguousarray(np.asarray(pos, dtype=np.float32))
    neigh = np.asarray(neigh_idx)
    W_rel = np.asarray(W_rel, dtype=np.float32)
    W_att = np.asarray(W_att, dtype=np.float32)
    W_glob = np.asarray(W_glob, dtype=np.float32)
    b_rel = np.asarray(b_rel, dtype=np.float32)
    b_glob = np.asarray(b_glob, dtype=np.float32)
    Wc, Wn, Wr, wd = W_rel[0:3], W_rel[3:6], W_rel[6:9], W_rel[9:10]

    nc = _get_nc()
    in_maps = [
        _prep_core(core, x, pos, neigh, Wc, Wn, Wr, wd, W_att, W_glob, b_rel, b_glob)
        for core in range(8)
    ]
    res = run_bass_kernel_spmd(nc, in_maps, core_ids=list(range(8)))
    out = np.zeros((B, C_OUT, N), np.float32)
    for core in range(8):
        b = core // 2
        P0 = (core % 2) * NP
        out[b, :, P0:P0 + NP] = res.results[core]["outp"]
    return out



# revision 3
# speedup vs baseline: 11.6850x; 11.6850x over previous
"""Trainium2 Bass kernel for the dense RandLA-Net block.

Reference computation (per batch b, point n, K=16 neighbors):
    enc   = [center(3), npos(3), rel(3), dist(1)]            # 10 dims
    rp    = relu(enc @ W_rel + b_rel)                        # 64
    f     = [rp, nfeat]                                      # 128
    att   = softmax_k(f @ W_att)                             # 128
    agg   = sum_k f * att                                    # 128
    out   = relu(agg @ W_glob + b_glob)                      # 128

Sharding: 8 cores = 4 batches x 2 point-halves (8192 points/core).

Data flow per core: a 256-byte token table in SBUF holds, per point,
its 64 feature channels (bf16) at words 0:64 and its position at words
64:67.  For each tile of 512 points x 16 k-slabs (8192 columns) one
SBUF-source transpose dma_gather materializes nfeat on partitions 0:64
and npos on partitions 64:67.  dist is computed from (npos-center) via
per-k-slab selector matmuls accumulating dist^2 into PSUM rows 0:16,
one sqrt, and a small transposing DMA back into partition row 67.  rp
is a 4-row matmul [npos;dist] plus a 3-row center matmul folded via
PSUM accumulation (rel is algebraically folded:  Wc*center + Wn*npos +
Wr*(npos-center) = (Wc-Wr)*center + (Wn+Wr)*npos).  The whole pipeline
runs in the "swapped" channel layout f = [nfeat(0:64); rp(64:128)],
handled by a permuted W_att and an un-permuting accumulation identity.
"""

import os
import sys

import numpy as np

sys.path.insert(0, "/opt/trn_rl_repo")

import ml_dtypes

import concourse.bass as bass
import concourse.tile as tile
from concourse import mybir, bacc
from concourse.bass_utils import run_bass_kernel_spmd

F32 = mybir.dt.float32
BF16 = mybir.dt.bfloat16
I16 = mybir.dt.int16
AF = mybir.ActivationFunctionType
OP = mybir.AluOpType
BF = ml_dtypes.bfloat16

B, C_IN, N, K = 4, 64, 16384, 16
D_REL, C_MID, C_OUT = 64, 128, 128
NP = N // 2            # points per core
PK = NP * K            # columns per core (131072)
NT = 16                # tiles (= point blocks of 512)
TC = 512               # points per tile
LT = TC * K            # 8192 cols per tile


def _build_kernel():
    nc = bacc.Bacc("TRN2", target_bir_lowering=False)

    xtab = nc.dram_tensor("xtab", [128, 16384], BF16, kind="ExternalInput")
    gidx = nc.dram_tensor("gidx", [128, 8192], I16, kind="ExternalInput")
    posc = nc.dram_tensor("posc", [4, NP], BF16, kind="ExternalInput")
    wcen = nc.dram_tensor("wcen", [128, 64], BF16, kind="ExternalInput")
    wnd = nc.dram_tensor("wnd", [128, 64], BF16, kind="ExternalInput")
    seld = nc.dram_tensor("seld", [128, 512], BF16, kind="ExternalInput")
    wattsw = nc.dram_tensor("wattsw", [128, 128], BF16, kind="ExternalInput")
    pswap = nc.dram_tensor("pswap", [128, 128], BF16, kind="ExternalInput")
    wglob = nc.dram_tensor("wglob", [128, 128], BF16, kind="ExternalInput")
    brel = nc.dram_tensor("brel", [128, 1], F32, kind="ExternalInput")
    bglob = nc.dram_tensor("bglob", [128, 1], F32, kind="ExternalInput")
    outp = nc.dram_tensor("outp", [128, NP], F32, kind="ExternalOutput")

    with tile.TileContext(nc) as tc:
        with tc.tile_pool(name="persist", bufs=1) as pp:
            xtab_sb = pp.tile([128, 16384], BF16)
            gidx_sb = pp.tile([128, 8192], I16)
            pos_sb = pp.tile([128, NP], BF16)
            wcen_sb = pp.tile([128, 64], BF16)
            wnd_sb = pp.tile([128, 64], BF16)
            seld_sb = pp.tile([128, 512], BF16)
            wattsw_sb = pp.tile([128, 128], BF16)
            pswap_sb = pp.tile([128, 128], BF16)
            wglob_sb = pp.tile([128, 128], BF16)
            brel_sb = pp.tile([128, 1], F32)
            bglob_sb = pp.tile([128, 1], F32)
            nc.sync.dma_start(out=xtab_sb, in_=xtab.ap())
            nc.sync.dma_start(out=gidx_sb, in_=gidx.ap())
            nc.sync.dma_start(out=pos_sb[64:68, :], in_=posc.ap())
            nc.sync.dma_start(out=wcen_sb, in_=wcen.ap())
            nc.sync.dma_start(out=wnd_sb, in_=wnd.ap())
            nc.sync.dma_start(out=seld_sb, in_=seld.ap())
            nc.sync.dma_start(out=wattsw_sb, in_=wattsw.ap())
            nc.sync.dma_start(out=pswap_sb, in_=pswap.ap())
            nc.sync.dma_start(out=wglob_sb, in_=wglob.ap())
            nc.sync.dma_start(out=brel_sb, in_=brel.ap())
            nc.sync.dma_start(out=bglob_sb, in_=bglob.ap())

            with tc.tile_pool(name="g", bufs=2) as gp, \
                 tc.tile_pool(name="work", bufs=2) as wp, \
                 tc.tile_pool(name="accps", bufs=1, space="PSUM") as psa, \
                 tc.tile_pool(name="mps", bufs=1, space="PSUM") as mps:
                for t in range(NT):
                    tcols = slice(t * TC, (t + 1) * TC)
                    g = gp.tile([128, LT], BF16, tag="g")
                    gout = bass.AP(tensor=g.tensor, offset=g.offset,
                                   ap=[[g.ap[0][0], 128], [LT, 1], [1, LT]])
                    nc.gpsimd.dma_gather(
                        out_ap=gout, in_ap=xtab_sb[:, :],
                        idxs_ap=gidx_sb[:, tcols],
                        num_idxs=LT, num_idxs_reg=LT, elem_size=128,
                        transpose=True,
                        sbuf_tokens_per_rank=128,
                        sbuf_free_dim_per_rank=256)

                    # rel = npos - center; m2 = rel^2 (in place)
                    cen = pos_sb[64:67, tcols]
                    cen_b = bass.AP(tensor=cen.tensor, offset=cen.offset,
                                    ap=[[cen.ap[0][0], 3], [0, 16], [1, TC]])
                    s_t = wp.tile([128, LT], BF16, tag="s")
                    nc.vector.tensor_tensor(
                        out=s_t[64:67, :].rearrange("p (k i) -> p k i", i=TC),
                        in0=g[64:67, :].rearrange("p (k i) -> p k i", i=TC),
                        in1=cen_b, op=OP.subtract)
                    nc.vector.tensor_mul(s_t[64:67, :], s_t[64:67, :],
                                         s_t[64:67, :])
                    # dist^2 per k-slab -> psd row k
                    psd = psa.tile([128, 512], F32, tag="psd")
                    for k in range(16):
                        nc.tensor.matmul(psd[0:32, :],
                                         seld_sb[64:67, 32 * k:32 * k + 32],
                                         s_t[64:67, k * 512:(k + 1) * 512],
                                         start=(k == 0), stop=(k == 15),
                                         tile_position=(64, 0),
                                         skip_group_check=True)
                    dsbT = wp.tile([128, 512], BF16, tag="dsbT")
                    nc.scalar.activation(out=dsbT[0:16, :], in_=psd[0:16, :],
                                         func=AF.Sqrt)
                    # scatter dist rows back into g[67, :]
                    drow = g[67:68, :]
                    drow = bass.AP(tensor=drow.tensor, offset=drow.offset,
                                   ap=[[drow.ap[0][0], 1], [512, 16], [1, 512]])
                    nc.sync.dma_start(out=drow, in_=dsbT[0:16, :])

                    ps_den = psa.tile([128, 512], F32, tag="den")
                    ps_num = psa.tile([128, 512], F32, tag="num")
                    for cc in range(16):
                        ccols = slice(cc * 512, (cc + 1) * 512)
                        ps_rp = mps.tile([128, 512], F32, tag="rp")
                        nc.tensor.matmul(ps_rp[64:128, :],
                                         wcen_sb[64:67, :],
                                         pos_sb[64:67, tcols],
                                         start=True, stop=False,
                                         tile_position=(64, 64),
                                         skip_group_check=True)
                        nc.tensor.matmul(ps_rp[64:128, :], wnd_sb[64:68, :],
                                         g[64:68, ccols],
                                         start=False, stop=True,
                                         tile_position=(64, 64),
                                         skip_group_check=True)
                        if cc % 2 == 0:
                            nc.scalar.activation(out=g[64:128, ccols],
                                                 in_=ps_rp[64:128, :],
                                                 func=AF.Relu,
                                                 bias=brel_sb[64:128, :],
                                                 scale=1.0)
                        else:
                            nc.vector.tensor_scalar(out=g[64:128, ccols],
                                                    in0=ps_rp[64:128, :],
                                                    scalar1=brel_sb[64:128, :],
                                                    scalar2=0.0,
                                                    op0=OP.add, op1=OP.max)
                        ps_s = mps.tile([128, 512], F32, tag="sc")
                        nc.tensor.matmul(ps_s, wattsw_sb, g[:, ccols],
                                         start=True, stop=True)
                        eu = wp.tile([128, 1024], BF16, tag="eu")
                        nc.scalar.activation(out=eu[:, 0:512], in_=ps_s,
                                             func=AF.Exp)
                        nc.vector.tensor_mul(eu[:, 512:1024], g[:, ccols],
                                             eu[:, 0:512])
                        nc.tensor.matmul(ps_den, pswap_sb, eu[:, 0:512],
                                         start=(cc == 0), stop=(cc == 15),
                                         skip_group_check=True)
                        nc.tensor.matmul(ps_num, pswap_sb, eu[:, 512:1024],
                                         start=(cc == 0), stop=(cc == 15),
                                         skip_group_check=True)
                    rcp = wp.tile([128, 512], F32, tag="rcp")
                    nc.vector.reciprocal(rcp, ps_den)
                    agg = wp.tile([128, 512], BF16, tag="agg")
                    nc.vector.tensor_mul(agg, ps_num, rcp)
                    ps_o = psa.tile([128, 512], F32, tag="o")
                    nc.tensor.matmul(ps_o, wglob_sb, agg, start=True, stop=True)
                    osb = wp.tile([128, 512], F32, tag="osb")
                    nc.scalar.activation(out=osb, in_=ps_o, func=AF.Relu,
                                         bias=bglob_sb, scale=1.0)
                    nc.sync.dma_start(out=outp.ap()[:, tcols], in_=osb)
    nc.compile()
    return nc


_NC = None


def _get_nc():
    global _NC
    if _NC is None:
        _NC = _build_kernel()
    return _NC


def _prep_core(core, x, pos, neigh, Wc, Wn, Wr, wd, W_att, W_glob, b_rel, b_glob):
    b = core // 2
    half = core % 2
    P0 = half * NP
    nb = neigh[b][P0:P0 + NP].astype(np.int64)      # [NP, K]
    xb = x[b]                                        # [64, N] f32
    posb = pos[b]                                    # [N, 3] f32

    # token table: token n -> partition n%128, rank n//128 (128 bf16 words)
    tok = np.zeros((N, 128), dtype=BF)
    tok[:, 0:64] = xb.T.astype(BF)
    tok[:, 64:67] = posb.astype(BF)
    xtab = np.ascontiguousarray(
        tok.reshape(128, 128, 128).transpose(1, 0, 2).reshape(128, 16384))

    # gather indices: tile t col j = k*512+i -> nb[t*512+i, k]; wrapped in 16
    # partitions (idx i at partition i%16, col i//16), replicated to 8 groups
    j = np.arange(LT)
    gidx = np.zeros((128, 8192), np.int16)
    for t in range(NT):
        nidx = nb[t * TC + (j & 511), j >> 9].astype(np.int16)
        blk = nidx.reshape(512, 16).T
        gidx[:, t * TC:(t + 1) * TC] = np.tile(blk, (8, 1))

    posc = np.zeros((4, NP), dtype=BF)
    posc[0:3] = posb[P0:P0 + NP].T.astype(BF)

    wcen_h = np.zeros((128, 64), dtype=BF)
    wcen_h[64:67] = (Wc - Wr).astype(BF)
    wnd_h = np.zeros((128, 64), dtype=BF)
    wnd_h[64:67] = (Wn + Wr).astype(BF)
    wnd_h[67] = wd[0].astype(BF)
    seld_h = np.zeros((128, 512), dtype=BF)
    for k in range(16):
        seld_h[64:67, 32 * k + k] = 1.0

    perm = (np.arange(128) + 64) % 128
    brel_h = np.zeros((128, 1), np.float32)
    brel_h[64:128, 0] = b_rel

    return {
        "xtab": xtab, "gidx": gidx, "posc": posc,
        "wcen": wcen_h, "wnd": wnd_h, "seld": seld_h,
        "wattsw": W_att[np.ix_(perm, perm)].astype(BF),
        "pswap": np.roll(np.eye(128, dtype=np.float32), 64, axis=0).astype(BF),
        "wglob": W_glob.astype(BF),
        "brel": brel_h,
        "bglob": b_glob.reshape(128, 1).astype(np.float32),
    }


def kernel(x, pos, neigh_idx, W_rel, b_rel, W_att, W_glob, b_glob, **kw):
    x = np.ascontiguousarray(np.asarray(x, dtype=np.float32))
    pos = np.ascontiguousarray(np.asarray(pos, dtype=np.float32))
    neigh = np.asarray(neigh_idx)
    W_rel = np.asarray(W_rel, dtype=np.float32)
    W_att = np.asarray(W_att, dtype=np.float32)
    W_glob = np.asarray(W_glob, dtype=np.float32)
    b_rel = np.asarray(b_rel, dtype=np.float32)
    b_glob = np.asarray(b_glob, dtype=np.float32)
    Wc, Wn, Wr, wd = W_rel[0:3], W_rel[3:6], W_rel[6:9], W_rel[9:10]

    nc = _get_nc()
    in_maps = [
        _prep_core(core, x, pos, neigh, Wc, Wn, Wr, wd, W_att, W_glob, b_rel, b_glob)
        for core in range(8)
    ]
    res = run_bass_kernel_spmd(nc, in_maps, core_ids=list(range(8)))
    out = np.zeros((B, C_OUT, N), np.float32)
    for core in range(8):
        b = core // 2
        P0 = (core % 2) * NP
        out[b, :, P0:P0 + NP] = res.results[core]["outp"]
    return out
